# revision 58
# baseline (speedup 1.0000x reference)
"""RWKV-v4 block (time-mix WKV attention + channel-mix GLU) on 8 Trainium2
NeuronCores, data-parallel over batch B.

v3: fp8 DoubleRow matmuls, weight-folded mixes, stage-skewed emission.

Layouts per core (B_local=4, T=1024, C=512, H=2048):
  - layout A: [t(128p), n(8), c(512)]  -- LN stats, residuals, store.
  - layout B pairs: [c(128p), pair(2), t] -- matmul operands (fp8 DoubleRow
    contracts 256 channels per instruction), WKV runs per 128-chn chunk.
  - A->B via bf16 DMA transpose through a DRAM bounce.

Mix folding: xk = xn + (tmk-1)*d with d = xn_t - xn_{t-1}, so
  k = [Wk*g1]@xn + [Wk*g1*(tmk-1)]@d  -- extra matmul accumulation steps
with host-folded weights instead of elementwise mix work (k/v/r and cWr).

WKV without max-tracking: |k| is small, so e = exp(k - 2) is safe in f32
and the constant shift cancels exactly in the N/D ratio:
  P_t = dly*P_{t-1} + e_t*v_t,  Q_t = dly*Q_{t-1} + e_t
  y_t = (P_{t-1} + eu*e_t*v_t) / (Q_{t-1} + eu*e_t)

Engine budget tricks:
  - LN stats via vector bn_stats/bn_aggr (one pass, off the scalar ACT).
  - sigmoid(x) = 0.5*(tanh(x/2)+1): Tanh lives in the same ACT table as
    Exp, killing per-chunk table reloads; the +1 rides the srw STT, the
    0.5 is folded into the Wo descale.
  - k/r/v/kk activations batched [128,1024] across two PSUM banks.
  - Stage-skewed emission: iteration i emits LN1(i) | WKV(i-1)+Wo/LN2(i-1)
    | CM(i-2) so in-order engines always have independent work queued.

fp8: weights scaled per-matrix to absmax 192 (e4m3 max finite 240);
descales ride existing post-matmul ops as [128,1] params.
"""

import numpy as np
import ml_dtypes
from contextlib import ExitStack

import concourse.bass as bass
import concourse.tile as tile
from concourse import bacc, mybir

B, T, C = 32, 1024, 512
H = 4 * C
NCORES = 8
BL = B // NCORES  # batches per core
NT = T // 128     # 8 t-subtiles per batch
CC = C // 128     # 4 channel chunks
NP = CC // 2      # 2 channel pair-chunks
HC = H // 128     # 16 hidden chunks
HP = HC // 2      # 8 hidden pair-chunks
TP = T + 32       # bounce rows (32-row zero pad for t-shift + xbar align)

F32 = mybir.dt.float32
BF16 = mybir.dt.bfloat16
F8 = mybir.dt.float8e4
AX = mybir.AxisListType
OP = mybir.AluOpType
AF = mybir.ActivationFunctionType
DR = mybir.MatmulPerfMode.DoubleRow

# w8a tile order (each [128, 2, 512] fp8):
#   kg0 kg1 kd0 kd1  vg0 vg1 vd0 vd1  rg0 rg1 rd0 rd1   (k/v/r folded)
#   wo0 wo1   cwv0..cwv7   crg0 crg1 crd0 crd1          (cWr folded)
NWA = 12 + 2 + 8 + 4
# fblob cols: delta[4] expu[4] cmkm1[4] ds_k ds_v ds_r/2 ds_wo/2 ds_cwk
#             ds_cwv ds_cwr
FCOLS = 12 + 7


def _emit(nc, tc, ctx, io, bl):
    x_d = io["x"].ap()
    y_d = io["y"].ap()

    sb = ctx.enter_context(tc.tile_pool(name="sb", bufs=1))
    psA = ctx.enter_context(tc.tile_pool(name="psA", bufs=2, space="PSUM"))
    pkk = ctx.enter_context(tc.tile_pool(name="pkk", bufs=2, space="PSUM"))
    dramp = ctx.enter_context(tc.tile_pool(name="dram", bufs=2, space="DRAM"))

    # ---- constants / weights (3 blob DMAs) ----
    w8a = sb.tile([128, NWA, 2, 512], F8, tag="w8a")
    nc.sync.dma_start(w8a[:], io["w8a"].ap())
    w8k = sb.tile([128, 2, 2, 2048], F8, tag="w8k")
    nc.sync.dma_start(w8k[:], io["w8k"].ap())
    fb = sb.tile([128, FCOLS], F32, tag="fb")
    nc.sync.dma_start(fb[:], io["fblob"].ap())

    def wa(i):
        return w8a[:, i, :, :]

    kg, kd = [wa(0), wa(1)], [wa(2), wa(3)]
    vg, vd = [wa(4), wa(5)], [wa(6), wa(7)]
    rg, rd = [wa(8), wa(9)], [wa(10), wa(11)]
    wo8 = [wa(12), wa(13)]
    cwv8 = [wa(14 + i) for i in range(8)]
    crg, crd = [wa(22), wa(23)], [wa(24), wa(25)]
    cwk8 = [w8k[:, i, :, :] for i in range(2)]

    delta_c = [fb[:, i:i + 1] for i in range(4)]
    expu_c = [fb[:, 4 + i:5 + i] for i in range(4)]
    cmkm1_c = [fb[:, 8 + i:9 + i] for i in range(4)]
    ds_k, ds_v, ds_rh, ds_woh, ds_cwk, ds_cwv, ds_cwr = (
        fb[:, 12 + i:13 + i] for i in range(7))

    eps_t = sb.tile([128, 1], F32, tag="eps")
    nc.vector.memset(eps_t[:], 1e-5)
    neg2_t = sb.tile([128, 1], F32, tag="neg2")
    nc.vector.memset(neg2_t[:], -2.0)
    zrow = sb.tile([32, C], BF16, tag="zrow")
    nc.vector.memset(zrow[:], 0.0)

    # ---- per-batch pools ----
    xa_pool = ctx.enter_context(tc.tile_pool(name="xa", bufs=3))
    x1_pool = ctx.enter_context(tc.tile_pool(name="x1", bufs=2))
    lnp = ctx.enter_context(tc.tile_pool(name="ln", bufs=2))
    bp = ctx.enter_context(tc.tile_pool(name="bp", bufs=2))
    bpB = ctx.enter_context(tc.tile_pool(name="bpB", bufs=1))
    wkvp = ctx.enter_context(tc.tile_pool(name="wkv", bufs=1))
    srwp = ctx.enter_context(tc.tile_pool(name="srw", bufs=2))
    cmp_ = ctx.enter_context(tc.tile_pool(name="cm", bufs=1))
    outp = ctx.enter_context(tc.tile_pool(name="out", bufs=2))

    xas = [None] * bl
    x1s = [None] * bl
    ln1 = [None] * bl   # (xn8, d8) layout-B products of LN1
    ln2 = [None] * bl   # (xn28, d28, xk28)
    srws = [None] * bl

    def g_layer_norm(src, which):
        """src [128, NT, 512] layout A -> normalized bf16 rows streamed
        straight to the DRAM bounce (per-n DMA; no full xn tile in SBUF).
        Stats via vector bn_stats; scalar does only Sqrt + per-n Identity."""
        bn6 = lnp.tile([128, NT, 6], F32, tag=f"bn6_{which}")
        mv = lnp.tile([128, NT, 2], F32, tag=f"mv_{which}")
        for n in range(NT):
            nc.vector.bn_stats(bn6[:, n, :], src[:, n, :])
            nc.vector.bn_aggr(mv[:, n, :], bn6[:, n, :])
        sqv = lnp.tile([128, NT], F32, tag=f"sqv_{which}")
        nc.scalar.activation(sqv[:], mv[:, :, 1], AF.Sqrt, bias=eps_t[:])
        rstd = lnp.tile([128, NT], F32, tag=f"rstd_{which}")
        nc.vector.reciprocal(rstd[:], sqv[:])
        mb = lnp.tile([128, NT], F32, tag=f"mb_{which}")
        nc.vector.scalar_tensor_tensor(mb[:], mv[:, :, 0], -1.0, rstd[:],
                                       op0=OP.mult, op1=OP.mult)
        yield
        xnd = dramp.tile([TP, C], BF16, tag="xnd")
        nc.sync.dma_start(xnd[0:32, :], zrow[:])
        for n in range(NT):
            xns = lnp.tile([128, 512], BF16, tag="xns", name=f"xns{n}")
            nc.scalar.activation(xns[:], src[:, n, :], AF.Identity,
                                 scale=rstd[:, n:n + 1], bias=mb[:, n:n + 1])
            nc.sync.dma_start(xnd[32 + n * 128:32 + (n + 1) * 128, :], xns[:])
        return xnd

    def bounce_to_B(xnd, which):
        """DRAM bounce -> fp8 pair tiles (xn8, d8) in layout B via DMA
        transpose (bf16), fp8 casts on GP."""
        xn8, d8 = [], []
        for pi in range(NP):
            t_ = bpB.tile([128, 2, TP], BF16, tag=f"xnB_{which}_{pi}")
            for i in range(2):
                cc = 2 * pi + i
                nc.sync.dma_start_transpose(
                    t_[:, i, :], xnd[:, cc * 128:(cc + 1) * 128])
            q_ = bp.tile([128, 2, T], F8, tag=f"xn8_{which}_{pi}")
            nc.scalar.activation(q_[:], t_[:, :, 32:TP], AF.Copy)
            xn8.append(q_)
            d_ = bp.tile([128, 2, T], F8, tag=f"d8_{which}_{pi}")
            nc.gpsimd.tensor_tensor(d_[:], t_[:, :, 32:TP], t_[:, :, 31:TP - 1],
                                    op=OP.subtract)
            d8.append(d_)
        return xn8, d8

    # ---------------- stages ----------------
    def P0(b):  # prefetch x
        xa = xa_pool.tile([128, NT, 512], F32, tag="xa")
        nc.sync.dma_start(xa[:], x_d[b].rearrange("(n p) c -> p n c", p=128))
        xas[b] = xa

    def g_P1(b):  # LN1 + bounce
        xnd = yield from g_layer_norm(xas[b], "a")
        yield
        ln1[b] = bounce_to_B(xnd, "a")

    def g_P2P3(b):  # k/v/r matmuls + WKV -> srw8 (fp8 pairs)
        xn8, d8 = ln1[b]
        srw8 = [srwp.tile([128, 2, T], F8, tag=f"srw_{pi}", name=f"srw_{pi}")
                for pi in range(NP)]
        srws[b] = srw8
        for hh in range(CC):
            hs = slice(hh * 128, (hh + 1) * 128)

            def mm_fold(wgp, wdp, tag):
                p_ = psA.tile([128, 2, 512], F32, tag="psA", name=f"p_{tag}{hh}")
                steps = ([(wgp[pi], xn8[pi], 0) for pi in range(NP)] +
                         [(wdp[pi], d8[pi], 0) for pi in range(NP)])
                for si, (w_, a_, off) in enumerate(steps):
                    for th in range(2):
                        nc.tensor.matmul(
                            p_[:, th, :], w_[:, :, hs],
                            a_[:, :, off + th * 512:off + (th + 1) * 512],
                            start=(si == 0), stop=(si == len(steps) - 1),
                            perf_mode=DR)
                return p_

            k_ps = mm_fold(kg, kd, "k")
            e = wkvp.tile([128, T], F32, tag="e")
            nc.scalar.activation(e[:], k_ps[:], AF.Exp, bias=neg2_t[:],
                                 scale=ds_k)
            r_ps = mm_fold(rg, rd, "r")
            tnh = wkvp.tile([128, T], BF16, tag="tnh")
            nc.scalar.activation(tnh[:], r_ps[:], AF.Tanh, scale=ds_rh)
            v_ps = mm_fold(vg, vd, "v")
            ev = wkvp.tile([128, T], F32, tag="ev")
            nc.vector.scalar_tensor_tensor(ev[:], v_ps[:], ds_v, e[:],
                                           op0=OP.mult, op1=OP.mult)
            Pb = wkvp.tile([128, T + 1], F32, tag="Pb")
            Qb = wkvp.tile([128, T + 1], F32, tag="Qb")
            nc.vector.memset(Pb[:, 0:1], 0.0)
            nc.gpsimd.memset(Qb[:, 0:1], 0.0)
            db = delta_c[hh].to_broadcast((128, T))
            nc.vector.tensor_tensor_scan(Pb[:, 1:T + 1], db, ev[:],
                                         0.0, op0=OP.mult, op1=OP.add)
            nc.vector.tensor_tensor_scan(Qb[:, 1:T + 1], db, e[:],
                                         0.0, op0=OP.mult, op1=OP.add)
            # N over ev, D over e (in place)
            nc.vector.scalar_tensor_tensor(ev[:], ev[:], expu_c[hh],
                                           Pb[:, 0:T], op0=OP.mult, op1=OP.add)
            nc.vector.scalar_tensor_tensor(e[:], e[:], expu_c[hh],
                                           Qb[:, 0:T], op0=OP.mult, op1=OP.add)
            rec = Qb[:, 0:T]
            nc.vector.reciprocal_approx_fast(rec, e[:])
            nc.gpsimd.tensor_tensor(ev[:], ev[:], rec, op=OP.mult)
            # srw = (tanh+1)*y = 2*sigmoid(r)*y; the 0.5 rides ds_wo
            nc.vector.scalar_tensor_tensor(srw8[hh // 2][:, hh % 2, :],
                                           tnh[:], 1.0, ev[:],
                                           op0=OP.add, op1=OP.mult)
            yield
        # ---- P3: Wo + residual -> x1 (bf16), LN2, bounce2, xk2 mix ----
        x1 = x1_pool.tile([128, NT, 512], BF16, tag="x1")
        x1s[b] = x1
        for half in range(4):
            n0 = 2 * half
            p_ = pkk.tile([128, 2, 512], F32, tag="pkk", name=f"p_wo{half}")
            for iq in range(2):
                for pi in range(NP):
                    n = n0 + iq
                    nc.tensor.matmul(p_[:, iq, :],
                                     srw8[pi][:, :, n * 128:(n + 1) * 128],
                                     wo8[pi][:], start=(pi == 0),
                                     stop=(pi == NP - 1), perf_mode=DR)
            nc.vector.scalar_tensor_tensor(x1[:, n0:n0 + 2, :], p_[:], ds_woh,
                                           xas[b][:, n0:n0 + 2, :],
                                           op0=OP.mult, op1=OP.add)
        yield
        xnd2 = yield from g_layer_norm(x1, "b")
        yield
        xn28, d28 = bounce_to_B(xnd2, "b")
        xk28 = []
        for pi in range(NP):
            t_ = bp.tile([128, 2, T], F8, tag=f"xk28_{pi}")
            for i in range(2):
                nc.vector.scalar_tensor_tensor(
                    t_[:, i, :], d28[pi][:, i, :], cmkm1_c[2 * pi + i],
                    xn28[pi][:, i, :], op0=OP.mult, op1=OP.add)
            xk28.append(t_)
        ln2[b] = (xn28, d28, xk28)

    def g_P4(b):  # channel mix -> y
        xn28, d28, xk28 = ln2[b]
        yb = y_d[b].rearrange("(n p) c -> p n c", p=128)
        for th in range(2):
            tsl = slice(th * 512, (th + 1) * 512)
            kk8 = cmp_.tile([128, HP, 2, 512], F8, tag="kk8")
            for hp in range(HP):
                kp = pkk.tile([128, 2, 512], F32, tag="pkk", name=f"kp{hp}")
                for i in range(2):
                    hh = 2 * hp + i
                    for pi in range(NP):
                        nc.tensor.matmul(
                            kp[:, i, :],
                            cwk8[pi][:, :, hh * 128:(hh + 1) * 128],
                            xk28[pi][:, :, tsl], start=(pi == 0),
                            stop=(pi == NP - 1), perf_mode=DR)
                rl = cmp_.tile([128, 2, 512], BF16, tag="rl")
                nc.scalar.activation(rl[:], kp[:], AF.Relu, scale=ds_cwk)
                nc.scalar.activation(kk8[:, hp], rl[:], AF.Square)
                if hp == 3:
                    yield
            yield
            for qp in range(2):
                n0 = th * 4 + 2 * qp
                rp = pkk.tile([128, 2, 512], F32, tag="pkk", name=f"rp{qp}")
                for iq in range(2):
                    n = n0 + iq
                    steps = ([(xn28[pi][:, :, n * 128:(n + 1) * 128],
                               crg[pi]) for pi in range(NP)] +
                             [(d28[pi][:, :, n * 128:(n + 1) * 128],
                               crd[pi]) for pi in range(NP)])
                    for si, (a_, w_) in enumerate(steps):
                        nc.tensor.matmul(rp[:, iq, :], a_, w_[:],
                                         start=(si == 0),
                                         stop=(si == len(steps) - 1),
                                         perf_mode=DR)
                sig2 = outp.tile([128, 2, 512], BF16, tag="sig2")
                nc.scalar.activation(sig2[:], rp[:], AF.Sigmoid, scale=ds_cwr)
                kvp = pkk.tile([128, 2, 512], F32, tag="pkk", name=f"kv{qp}")
                for iq in range(2):
                    q = 2 * qp + iq
                    for hp in range(HP):
                        nc.tensor.matmul(
                            kvp[:, iq, :], kk8[:, hp, :, q * 128:(q + 1) * 128],
                            cwv8[hp][:], start=(hp == 0), stop=(hp == HP - 1),
                            perf_mode=DR)
                t2 = outp.tile([128, 2, 512], F32, tag="t2")
                nc.vector.scalar_tensor_tensor(t2[:], kvp[:], ds_cwv, sig2[:],
                                               op0=OP.mult, op1=OP.mult)
                nc.gpsimd.tensor_tensor(t2[:], t2[:], x1s[b][:, n0:n0 + 2, :],
                                        op=OP.add)
                nc.sync.dma_start(yb[:, n0:n0 + 2, :], t2[:])
                yield

    # ---------------- stage-skewed, chunk-interleaved schedule ----------
    # Iteration i co-emits WKV/Wo/LN2(i-1), CM(i-2) and LN1(i) round-robin
    # at ~10us chunk granularity so every in-order engine queue always
    # holds independent work from a neighboring batch.
    P0(0)
    for i in range(bl + 2):
        if i + 1 < bl:
            P0(i + 1)
        gens = []
        if 1 <= i <= bl:
            gens.append(g_P2P3(i - 1))
        if i < bl:
            gens.append(g_P1(i))
        if i >= 2:
            gens.append(g_P4(i - 2))
        while gens:
            alive = []
            for g in gens:
                try:
                    next(g)
                    alive.append(g)
                except StopIteration:
                    pass
            gens = alive


def build_program(bl=BL):
    nc = bacc.Bacc("TRN2", target_bir_lowering=False, debug=False,
                   num_devices=NCORES)
    io = {}
    io["x"] = nc.dram_tensor("x", [bl, T, C], F32, kind="ExternalInput")
    io["y"] = nc.dram_tensor("y", [bl, T, C], F32, kind="ExternalOutput")
    io["w8a"] = nc.dram_tensor("w8a", [128, NWA, 2, 512], F8,
                               kind="ExternalInput")
    io["w8k"] = nc.dram_tensor("w8k", [128, 2, 2, 2048], F8,
                               kind="ExternalInput")
    io["fblob"] = nc.dram_tensor("fblob", [128, FCOLS], F32,
                                 kind="ExternalInput")

    with tile.TileContext(nc) as tc:
        with ExitStack() as ctx:
            _emit(nc, tc, ctx, io, bl)
    nc.compile()
    return nc


def host_params(inputs):
    """Host-side parameter prep (O(C^2) only): fold LN gamma + mix coefs
    into fp8 weights, pack pair layouts, compute scan constants."""
    f32 = np.float32
    f8 = ml_dtypes.float8_e4m3
    g1 = np.asarray(inputs["ln1_g"], f32)
    b1 = np.asarray(inputs["ln1_b"], f32)
    g2 = np.asarray(inputs["ln2_g"], f32)
    b2 = np.asarray(inputs["ln2_b"], f32)
    assert np.allclose(b1, 0.0) and np.allclose(b2, 0.0), \
        "nonzero LN bias not supported"
    Wk = np.asarray(inputs["Wk"], f32)
    Wv = np.asarray(inputs["Wv"], f32)
    Wr = np.asarray(inputs["Wr"], f32)
    Wo = np.asarray(inputs["Wo"], f32)
    cWk = np.asarray(inputs["cWk"], f32)
    cWr = np.asarray(inputs["cWr"], f32)
    cWv = np.asarray(inputs["cWv"], f32)
    tmk = np.asarray(inputs["tm_k"], f32)
    tmv = np.asarray(inputs["tm_v"], f32)
    tmr = np.asarray(inputs["tm_r"], f32)
    cmk = np.asarray(inputs["cm_k"], f32)
    cmr = np.asarray(inputs["cm_r"], f32)

    def pairs(MT):
        # MT [Cin, Cout] f32 -> list over Cin 256-pairs of [128, 2, Cout]
        ci, co = MT.shape
        return [np.stack([MT[256 * pi:256 * pi + 128],
                          MT[256 * pi + 128:256 * pi + 256]], axis=1)
                for pi in range(ci // 256)]

    def scaled(*mats):
        amax = max(np.abs(m).max() for m in mats)
        s = 192.0 / max(amax, 1e-30)
        return s, [np.ascontiguousarray((p * s).astype(f8))
                   for m in mats for p in pairs(m)]

    kgT = (Wk * g1).T
    kdT = (Wk * (g1 * (tmk - 1.0))).T
    vgT = (Wv * g1).T
    vdT = (Wv * (g1 * (tmv - 1.0))).T
    rgT = (Wr * g1).T
    rdT = (Wr * (g1 * (tmr - 1.0))).T
    s_k, k8 = scaled(kgT, kdT)
    s_v, v8 = scaled(vgT, vdT)
    s_r, r8 = scaled(rgT, rdT)
    s_wo, wo8 = scaled(Wo.T)
    s_cwk, cwk8 = scaled((cWk * g2).T)       # [C, H]: 2 pairs of [128,2,2048]
    s_cwv, cwv8 = scaled(cWv.T)              # cWvT [H,C]: 8 pairs over H
    crgT = (cWr * g2).T
    crdT = (cWr * (g2 * (cmr - 1.0))).T
    s_cr, cr8 = scaled(crgT, crdT)

    w8a = np.stack(k8 + v8 + r8 + wo8 + cwv8 + cr8, axis=1)
    assert w8a.shape == (128, NWA, 2, 512), w8a.shape
    w8k = np.stack(cwk8, axis=1)
    assert w8k.shape == (128, 2, 2, 2048), w8k.shape

    fblob = np.zeros((128, FCOLS), f32)
    delta = np.exp(-np.exp(np.asarray(inputs["time_decay"], f32)))
    expu = np.exp(np.asarray(inputs["time_first"], f32))
    for i in range(4):
        fblob[:, i] = delta[i * 128:(i + 1) * 128]
        fblob[:, 4 + i] = expu[i * 128:(i + 1) * 128]
        fblob[:, 8 + i] = cmk[i * 128:(i + 1) * 128] - 1.0
    # tanh trick: sigmoid(x) = 0.5*(tanh(x/2)+1) -> r descale halved for
    # the tanh scale; the 0.5 from (tanh+1) folded into the Wo descale.
    ds = [1.0 / s_k, 1.0 / s_v, 0.5 / s_r, 0.5 / s_wo, 1.0 / s_cwk,
          1.0 / s_cwv, 1.0 / s_cr]
    for j, v in enumerate(ds):
        fblob[:, 12 + j] = v

    return {"w8a": w8a, "w8k": w8k, "fblob": fblob}


_CACHE = {}


def kernel(**inputs):
    from concourse.bass_utils import run_bass_kernel_spmd

    if "nc" not in _CACHE:
        _CACHE["nc"] = build_program(BL)
    nc = _CACHE["nc"]

    p = host_params(inputs)
    x = np.asarray(inputs["x"], np.float32)
    in_maps = []
    for c in range(NCORES):
        m = dict(p)
        m["x"] = np.ascontiguousarray(x[c * BL:(c + 1) * BL])
        in_maps.append(m)
    res = run_bass_kernel_spmd(nc, in_maps, list(range(NCORES)))
    out = np.concatenate([res.results[c]["y"] for c in range(NCORES)], axis=0)
    return out.astype(np.float32)
